# revision 30
# baseline (speedup 1.0000x reference)
"""Trainium2 Bass kernel for nn_PolicyNetwork (dense_mlp, 8-core data-parallel).

Reference computation:
  op branch:    ops = softmax_global( MLP3([x, y[dag], z]) - 1000*(1-op_msk) )
  prlvl branch: prlvl = softmax_rows( MLP3([limits, y, z]) - 1000*(1-prlvl_msk) )

Sharding: x/op_msk split along rows (12800/core), y/prlvl_msk along dags
(128/core); weights/z replicated.  The global op softmax uses an AllGather of
per-core (max, sumexp) partials.

Device-side math per core (all fp32):
  - x tiles [128,256] are PE-transposed (identity matmul) into xT strips
    [128 feat, rows]; layer 1 runs feature-major: h1T = W1x^T-chunks @ xT,
    dag-bias (y@W1y + z@W1z + b1) folded in via ACT relu bias (per-partition,
    dag-packed 4x32 layout).
  - layers 2/3 use block-diagonal weights (4x/8x) so 4/8 dags are processed
    per matmul; layer-3 outputs land as a [128 dags, 100 ops] PSUM tile.
  - b3 is constant across the softmax axis in both branches -> dropped.
"""

import numpy as np

import concourse.bass as bass
import concourse.tile as tile
from concourse import bacc, bass_isa, mybir
from concourse.bass_utils import run_bass_kernel_spmd

FP = mybir.dt.float32
E = 256
D_TOT = 1024
W_WORKERS = 64
OPS = 100
H1, H2 = 32, 16
N_CORES = 8
DC_FULL = D_TOT // N_CORES      # 128 dags per core
GRP_TILES_FULL = 4              # 128-row tiles per x DMA/transpose group


def _dag_pieces(d, grpw):
    """Column pieces of local dag d's rows [100d, 100d+100) split at xT-group
    boundaries (group width grpw cols). Returns (group, off_in_group, width,
    col_base_in_dag)."""
    c0, c1 = OPS * d, OPS * d + OPS
    out = []
    g0, g1 = c0 // grpw, (c1 - 1) // grpw
    for g in range(g0, g1 + 1):
        lo, hi = max(c0, grpw * g), min(c1, grpw * g + grpw)
        out.append((g, lo - grpw * g, hi - lo, lo - c0))
    return out


def emit(tc, io, dc, grp_tiles, n_cores):
    from contextlib import ExitStack
    ctx = ExitStack()
    nc = tc.nc
    rows = dc * OPS
    n_tiles = rows // 128
    assert n_tiles * 128 == rows and n_tiles % grp_tiles == 0
    n_groups = n_tiles // grp_tiles
    grpw = 128 * grp_tiles          # xT cols per chunk per group
    nj = dc // 4                    # L1 relu groups (4 dags each)
    nj2 = dc // 8                   # L2 relu groups (8 dags each)

    cpool = ctx.enter_context(tc.tile_pool(name="consts", bufs=1))
    dpool = ctx.enter_context(tc.tile_pool(name="dram", bufs=1, space="DRAM"))

    def ptr(out, in_, idn, start, stop):
        # PE transpose with explicit PSUM accumulation flags
        nc.tensor.matmul(out, in_, idn, is_transpose=True, start=start,
                         stop=stop, skip_group_check=True)

    def cload(name, shape, src_ap=None):
        t = cpool.tile(shape, FP, tag=name)
        nc.sync.dma_start(t[:], io[name][:] if src_ap is None else src_ap)
        return t

    ident = cload("ident", [128, 128])
    ones1 = cload("ones1", [1, 128])

    # first-layer weights: [256,32] -> [128, 2x32] (chunk k at cols 32k)
    def wchunks(name):
        t = cpool.tile([128, 64], FP, tag=name)
        nc.sync.dma_start(t[:].rearrange("p (k h) -> p k h", k=2),
                          io[name].rearrange("(k p) h -> p k h", k=2))
        return t

    w1x = wchunks("op_w1x")
    w1y = wchunks("op_w1y")
    w1yp = wchunks("pr_w1y")
    czop = cload("czb1op", [1, 32])
    czpr = cload("czb1pr", [1, 32])
    w2ob = cload("op_w2blk", [128, 64])
    w3ob = cload("op_w3blk", [128, 8])
    b2ot = cload("op_b2tile", [128, 1])
    w2pb = cload("pr_w2blk", [128, 64])
    w3pb = cload("pr_w3blk", [128, 8])
    b2pt = cload("pr_b2tile", [128, 1])
    lwp = cload("lwpack", [128, 64])
    y_sb = cload("y_sh", [dc, E])
    opm = cload("opm_sh", [dc, OPS])
    prm = cload("prm_sh", [dc, W_WORKERS])

    spool = ctx.enter_context(tc.tile_pool(name="sb", bufs=1))

    # ---- yT + dag-bias vectors c1 = y@W1y + (z@W1z + b1) --------------------
    with tc.tile_pool(name="ps_pre", bufs=1, space="PSUM") as psa:
        # yT chunks: [128 feat, dc dags], chunk k at free offset dc*k
        ypt = psa.tile([128, 2 * dc], FP, tag="ypt")
        seen = set()
        for k in range(2):
            off = dc * k * 4
            nc.tensor.matmul(ypt[:, dc * k:dc * k + dc],
                             y_sb[:, 128 * k:128 * k + 128],
                             ident[:dc, :dc], is_transpose=True,
                             start=(off // 2048 not in seen), stop=(k == 1),
                             skip_group_check=True)
            seen.add(off // 2048)
        yts = spool.tile([128, 2 * dc], FP, tag="yts")
        nc.scalar.copy(yts[:], ypt[:])

        c1_sb = {}
        for nm, wch, cz in (("op", w1y, czop), ("pr", w1yp, czpr)):
            c1p = psa.tile([dc, 32], FP, tag=f"c1{nm}p")
            for k in range(2):
                nc.tensor.matmul(c1p[:], yts[:, dc * k:dc * k + dc],
                                 wch[:, 32 * k:32 * k + 32],
                                 start=(k == 0), stop=False)
            nc.tensor.matmul(c1p[:], ones1[:1, :dc], cz[:],
                             start=False, stop=True)
            sb = spool.tile([dc, 32], FP, tag=f"c1{nm}sb")
            nc.scalar.copy(sb[:], c1p[:])
            c1_sb[nm] = sb

    # rearrange c1 [dc,32] -> pack [(g,h), j] = c1[4j+g, h] via DRAM bounce
    c1pack = {}
    for nm in ("op", "pr"):
        bnc = dpool.tile([dc, 32], FP, tag=f"bnc_{nm}")
        nc.sync.dma_start(bnc[:], c1_sb[nm][:])
        pk = spool.tile([128, nj], FP, tag=f"c1{nm}pack")
        nc.sync.dma_start(pk[:], bnc[:].rearrange("(j g) h -> (g h) j", g=4))
        c1pack[nm] = pk

    # ---- prlvl branch ------------------------------------------------------
    with tc.tile_pool(name="ps_pr", bufs=2, space="PSUM") as psp, \
         tc.tile_pool(name="ps_prl", bufs=1, space="PSUM") as pspl:
        h1p = []
        for j in range(nj):
            t = spool.tile([128, W_WORKERS], FP, tag=f"h1p{j % 8}")
            nc.scalar.activation(t[:], lwp[:],
                                 mybir.ActivationFunctionType.Relu,
                                 bias=c1pack["pr"][:, j:j + 1])
            h1p.append(t)
        h2p = []
        for j2 in range(nj2):
            l2pp = psp.tile([128, W_WORKERS], FP, tag="l2pp")
            for jj in range(2):
                nc.tensor.matmul(l2pp[64 * jj:64 * jj + 64, :], w2pb[:],
                                 h1p[2 * j2 + jj][:], start=True, stop=True,
                                 skip_group_check=True)
            t = spool.tile([128, W_WORKERS], FP, tag=f"h2p{j2 % 4}")
            nc.scalar.activation(t[:], l2pp[:],
                                 mybir.ActivationFunctionType.Relu,
                                 bias=b2pt[:, 0:1])
            h2p.append(t)
        # L3 transposed: lpT[w, dag] accumulates 16 [64,8] slices + mask.T
        prmadj = spool.tile([dc, W_WORKERS], FP, tag="prmadj")
        nc.vector.tensor_scalar(prmadj[:], prm[:], 1000.0, -1000.0,
                                op0=mybir.AluOpType.mult,
                                op1=mybir.AluOpType.add)
        lpT = pspl.tile([W_WORKERS, dc], FP, tag="lpT")
        for j2 in range(nj2):
            nc.tensor.matmul(lpT[:, 8 * j2:8 * j2 + 8], h2p[j2][:], w3pb[:],
                             start=(j2 == 0), stop=False,
                             skip_group_check=True)
        nc.tensor.matmul(lpT[:], prmadj[:], ident[:dc, :dc],
                         start=False, stop=True, skip_group_check=True)
        lpts = spool.tile([W_WORKERS, dc], FP, tag="lpts")
        nc.scalar.copy(lpts[:], lpT[:])
        lpm = psp.tile([dc, W_WORKERS], FP, tag="l2pp")
        nc.tensor.matmul(lpm[:], lpts[:], ident[:W_WORKERS, :W_WORKERS],
                         start=True, stop=True)
        nmx = spool.tile([dc, 1], FP, tag="nmx")
        nc.vector.tensor_reduce(nmx[:], lpm[:], axis=mybir.AxisListType.X,
                                op=mybir.AluOpType.max, negate=True)
        ep = spool.tile([dc, W_WORKERS], FP, tag="ep")
        sep = spool.tile([dc, 1], FP, tag="sep")
        nc.scalar.activation(ep[:], lpm[:], mybir.ActivationFunctionType.Exp,
                             bias=nmx[:, 0:1], accum_out=sep[:])
        rp = spool.tile([dc, 1], FP, tag="rp")
        nc.vector.reciprocal(rp[:], sep[:])
        pro = spool.tile([dc, W_WORKERS], FP, tag="pro")
        nc.vector.tensor_scalar_mul(pro[:], ep[:], rp[:, 0:1])
        nc.sync.dma_start(io["prlvl_out"][:], pro[:])

    # ---- op branch: transpose x, 3-layer MLP, masked global softmax --------
    x_re = io["x_sh"].rearrange("(g jj p) e -> g p jj e", jj=grp_tiles, p=128)
    xts = []
    xt_bufs = 2 if 2 * grpw * 4 <= 4096 else 1
    with tc.tile_pool(name="ps_xt", bufs=xt_bufs, space="PSUM") as psxt, \
         tc.tile_pool(name="xin", bufs=3) as xpool, \
         tc.tile_pool(name="xts", bufs=4) as xtspool, \
         tc.tile_pool(name="ps_l1", bufs=2, space="PSUM") as psl1, \
         tc.tile_pool(name="ps_l2", bufs=1, space="PSUM") as psl2, \
         tc.tile_pool(name="ps_lg", bufs=1, space="PSUM") as pslg:
        for g in range(n_groups):
            x4 = xpool.tile([128, grp_tiles, E], FP, tag="x4")
            nc.sync.dma_start(x4[:], x_re[g])
            xtp = psxt.tile([128, 2 * grpw], FP, tag="xtp")
            seen = set()
            for jj in range(grp_tiles):
                for k in range(2):
                    bank = (grpw * k + 128 * jj) * 4 // 2048
                    ptr(xtp[:, grpw * k + 128 * jj: grpw * k + 128 * jj + 128],
                        x4[:, jj, 128 * k:128 * k + 128], ident[:],
                        start=(bank not in seen),
                        stop=(jj == grp_tiles - 1 and k == 1))
                    seen.add(bank)
            xt = xtspool.tile([128, 2 * grpw], FP, tag="xt")
            nc.vector.tensor_copy(xt[:, 0:grpw], xtp[:, 0:grpw])
            nc.scalar.copy(xt[:, grpw:2 * grpw], xtp[:, grpw:2 * grpw])
            xts.append(xt)

        h1o = []
        for j in range(nj):
            l1pt = psl1.tile([128, 512], FP, tag="l1p")
            l1p = l1pt[:, 0:OPS]
            for k in range(2):
                for d in range(4 * j, 4 * j + 4):
                    np_d = len(_dag_pieces(d, grpw))
                    for pi, (g, off, wid, cb) in enumerate(_dag_pieces(d, grpw)):
                        nc.tensor.matmul(
                            l1p[32 * (d % 4):32 * (d % 4) + 32, cb:cb + wid],
                            w1x[:, 32 * k:32 * k + 32],
                            xts[g][:, grpw * k + off: grpw * k + off + wid],
                            start=(k == 0 and pi == 0),
                            stop=(k == 1 and pi == np_d - 1),
                            skip_group_check=True,
                            tile_position=(0, 32 * (d % 4)))
            t = spool.tile([128, OPS], FP, tag=f"h1o{j % 8}")
            nc.scalar.activation(t[:], l1p[:],
                                 mybir.ActivationFunctionType.Relu,
                                 bias=c1pack["op"][:, j:j + 1])
            h1o.append(t)

        h2o = []
        for j2 in range(nj2):
            l2p = psl2.tile([128, OPS], FP, tag="l2p")
            for jj in range(2):
                nc.tensor.matmul(l2p[64 * jj:64 * jj + 64, :], w2ob[:],
                                 h1o[2 * j2 + jj][:], start=True, stop=True,
                                 skip_group_check=True)
            t = spool.tile([128, OPS], FP, tag=f"h2o{j2 % 4}")
            nc.scalar.activation(t[:], l2p[:],
                                 mybir.ActivationFunctionType.Relu,
                                 bias=b2ot[:, 0:1])
            h2o.append(t)

        opmadj = spool.tile([dc, OPS], FP, tag="opmadj")
        nc.vector.tensor_scalar(opmadj[:], opm[:], 1000.0, -1000.0,
                                op0=mybir.AluOpType.mult,
                                op1=mybir.AluOpType.add)
        lgT = pslg.tile([OPS, dc], FP, tag="lgT")
        for j2 in range(nj2):
            nc.tensor.matmul(lgT[:, 8 * j2:8 * j2 + 8], h2o[j2][:], w3ob[:],
                             start=(j2 == 0), stop=False,
                             skip_group_check=True)
        nc.tensor.matmul(lgT[:], opmadj[:], ident[:dc, :dc],
                         start=False, stop=True, skip_group_check=True)
        lgts = spool.tile([OPS, dc], FP, tag="lgts")
        nc.scalar.copy(lgts[:], lgT[:])
        lomt = psl1.tile([128, 512], FP, tag="l1p")
        lom = lomt[0:dc, 0:OPS]
        nc.tensor.matmul(lom, lgts[:], ident[:OPS, :OPS],
                         start=True, stop=True)

        mx = spool.tile([dc, 1], FP, tag="mx")
        nc.vector.tensor_reduce(mx[:], lom, axis=mybir.AxisListType.X,
                                op=mybir.AluOpType.max)
        mcb = spool.tile([dc, 1], FP, tag="mcb")
        nc.gpsimd.partition_all_reduce(mcb[:], mx[:], dc,
                                       bass_isa.ReduceOp.max)
        mcnb = spool.tile([dc, 1], FP, tag="mcnb")
        nc.vector.tensor_scalar_mul(mcnb[:], mcb[:], -1.0)
        eo = spool.tile([dc, OPS], FP, tag="eo")
        se = spool.tile([dc, 1], FP, tag="se")
        nc.scalar.activation(eo[:], lom, mybir.ActivationFunctionType.Exp,
                             bias=mcnb[:, 0:1], accum_out=se[:])
        scb = spool.tile([dc, 1], FP, tag="scb")
        nc.gpsimd.partition_all_reduce(scb[:], se[:], dc,
                                       bass_isa.ReduceOp.add)

        # cross-core: AllGather per-core (max, sumexp)
        part = spool.tile([1, 2], FP, tag="part")
        nc.vector.tensor_copy(part[0:1, 0:1], mcb[0:1, 0:1])
        nc.vector.tensor_copy(part[0:1, 1:2], scb[0:1, 0:1])
        ag_in = dpool.tile([1, 2], FP, tag="ag_in")
        ag_out = dpool.tile([1, 2 * n_cores], FP, tag="ag_out")
        nc.sync.dma_start(ag_in[:], part[:])
        nc.gpsimd.collective_compute(
            "AllGather", mybir.AluOpType.bypass,
            replica_groups=[list(range(n_cores))],
            ins=[ag_in.opt()], outs=[ag_out.opt()])
        gth = spool.tile([1, 2 * n_cores], FP, tag="gth")
        nc.sync.dma_start(gth[:], ag_out[:])

        gv = gth[:].rearrange("p (c t) -> p t c", t=2)  # [1, 2, n_cores]
        nM = spool.tile([1, 1], FP, tag="nM")
        nc.vector.tensor_reduce(nM[:], gv[:, 0:1, :],
                                axis=mybir.AxisListType.X,
                                op=mybir.AluOpType.max, negate=True)
        em = spool.tile([1, n_cores], FP, tag="em")
        nc.scalar.activation(em[:], gv[:, 0, :],
                             mybir.ActivationFunctionType.Exp,
                             bias=nM[:, 0:1])
        zs = spool.tile([1, n_cores], FP, tag="zs")
        nc.vector.tensor_mul(zs[:], em[:], gv[:, 1, :])
        zt = spool.tile([1, 1], FP, tag="zt")
        nc.vector.tensor_reduce(zt[:], zs[:], axis=mybir.AxisListType.X,
                                op=mybir.AluOpType.add)
        am = spool.tile([1, 1], FP, tag="am")
        nc.scalar.activation(am[:], mcb[0:1, 0:1],
                             mybir.ActivationFunctionType.Exp,
                             bias=nM[:, 0:1])
        zi = spool.tile([1, 1], FP, tag="zi")
        nc.vector.reciprocal(zi[:], zt[:])
        alpha = spool.tile([1, 1], FP, tag="alpha")
        nc.vector.tensor_mul(alpha[:], am[:], zi[:])
        alphab = spool.tile([dc, 1], FP, tag="alphab")
        nc.gpsimd.partition_broadcast(alphab[:], alpha[:])
        oout = spool.tile([dc, OPS], FP, tag="oout")
        nc.vector.tensor_scalar_mul(oout[:], eo[:], alphab[:, 0:1])
        nc.sync.dma_start(io["ops_out"][:], oout[:])
    ctx.close()


def build(dc=DC_FULL, grp_tiles=GRP_TILES_FULL, n_cores=N_CORES):
    rows = dc * OPS
    nc = bacc.Bacc("TRN2", target_bir_lowering=False, debug=False,
                   num_devices=n_cores)
    names_in = {
        "ident": [128, 128], "ones1": [1, 128],
        "op_w1x": [E, H1], "op_w1y": [E, H1], "pr_w1y": [E, H1],
        "czb1op": [1, H1], "czb1pr": [1, H1],
        "op_w2blk": [128, 64], "op_w3blk": [128, 8], "op_b2tile": [128, 1],
        "pr_w2blk": [128, 64], "pr_w3blk": [128, 8], "pr_b2tile": [128, 1],
        "lwpack": [128, W_WORKERS],
        "x_sh": [rows, E], "y_sh": [dc, E],
        "opm_sh": [dc, OPS], "prm_sh": [dc, W_WORKERS],
    }
    io = {}
    for nm, shp in names_in.items():
        io[nm] = nc.dram_tensor(nm, shp, FP, kind="ExternalInput").ap()
    io["ops_out"] = nc.dram_tensor("ops_out", [dc, OPS], FP,
                                   kind="ExternalOutput").ap()
    io["prlvl_out"] = nc.dram_tensor("prlvl_out", [dc, W_WORKERS], FP,
                                     kind="ExternalOutput").ap()
    with tile.TileContext(nc) as tc:
        emit(tc, io, dc, grp_tiles, n_cores)
    nc.compile()
    return nc


def make_const_inputs(op_W1, op_b1, op_W2, op_b2, op_W3,
                      pr_W1, pr_b1, pr_W2, pr_b2, pr_W3, z):
    """Host-side constant prep (tiny numpy)."""
    f = np.float32

    def blk4(w2):  # [32,16] -> [128,64] block-diag x4
        out = np.zeros((128, 64), f)
        for g in range(4):
            out[32 * g:32 * g + 32, 16 * g:16 * g + 16] = w2
        return out

    def blk8(w3):  # [16,1] -> [128,8]; partition (jj,g,h2) -> dag 4jj+g
        out = np.zeros((128, 8), f)
        for p in range(128):
            jj, g, h2 = p // 64, (p % 64) // 16, p % 16
            out[p, 4 * jj + g] = w3[h2, 0]
        return out

    lw = np.outer(pr_W1[0], np.arange(1, W_WORKERS + 1, dtype=f))  # [32,64]
    return {
        "ident": np.eye(128, dtype=f),
        "ones1": np.ones((1, 128), f),
        "op_w1x": np.ascontiguousarray(op_W1[0:E], f),
        "op_w1y": np.ascontiguousarray(op_W1[E:2 * E], f),
        "pr_w1y": np.ascontiguousarray(pr_W1[1:1 + E], f),
        "czb1op": (z[0] @ op_W1[2 * E:3 * E] + op_b1).reshape(1, H1).astype(f),
        "czb1pr": (z[0] @ pr_W1[1 + E:1 + 2 * E] + pr_b1).reshape(1, H1).astype(f),
        "op_w2blk": blk4(np.asarray(op_W2, f)),
        "op_w3blk": blk8(np.asarray(op_W3, f)),
        "op_b2tile": np.tile(np.asarray(op_b2, f), 8).reshape(128, 1),
        "pr_w2blk": blk4(np.asarray(pr_W2, f)),
        "pr_w3blk": blk8(np.asarray(pr_W3, f)),
        "pr_b2tile": np.tile(np.asarray(pr_b2, f), 8).reshape(128, 1),
        "lwpack": np.tile(lw, (4, 1)).astype(f),
    }


_compiled = {}
TRACE = False           # set by test harness to capture NTFF profile
LAST_RESULTS = None     # BassKernelResults of the last kernel() run


def _get_compiled(key=("full",)):
    if key not in _compiled:
        _compiled[key] = build()
    return _compiled[key]


def kernel(num_ops, num_dags, num_workers, x, y, z, op_msk, prlvl_msk,
           op_W1, op_b1, op_W2, op_b2, op_W3, op_b3,
           pr_W1, pr_b1, pr_W2, pr_b2, pr_W3, pr_b3):
    x = np.asarray(x, np.float32)
    y = np.asarray(y, np.float32)
    z = np.asarray(z, np.float32)
    op_msk = np.asarray(op_msk, np.float32)
    prlvl_msk = np.asarray(prlvl_msk, np.float32)
    consts = make_const_inputs(
        np.asarray(op_W1), np.asarray(op_b1), np.asarray(op_W2),
        np.asarray(op_b2), np.asarray(op_W3),
        np.asarray(pr_W1), np.asarray(pr_b1), np.asarray(pr_W2),
        np.asarray(pr_b2), np.asarray(pr_W3), z)

    nc = _get_compiled()
    rows = DC_FULL * OPS
    in_maps = []
    for c in range(N_CORES):
        m = dict(consts)
        m["x_sh"] = x[c * rows:(c + 1) * rows]
        m["y_sh"] = y[c * DC_FULL:(c + 1) * DC_FULL]
        m["opm_sh"] = op_msk[c * rows:(c + 1) * rows].reshape(DC_FULL, OPS)
        m["prm_sh"] = prlvl_msk[c * DC_FULL:(c + 1) * DC_FULL]
        in_maps.append(m)

    global LAST_RESULTS
    res = run_bass_kernel_spmd(nc, in_maps, list(range(N_CORES)),
                               trace=TRACE)
    LAST_RESULTS = res
    ops = np.concatenate(
        [res.results[c]["ops_out"].reshape(-1) for c in range(N_CORES)])
    prlvl = np.concatenate(
        [res.results[c]["prlvl_out"] for c in range(N_CORES)], axis=0)
    return ops, prlvl
